# revision 24
# baseline (speedup 1.0000x reference)
"""LIF layer (T=64, B=128, 2048->2048) on 8 trn2 NeuronCores.

Sharding: 2-way over out_dim x 4-way over batch. Each core owns
O_loc=1024 output channels (8 stationary chunks of 128) and B_loc=32
batch rows, so per-core HBM traffic is x 16MB + W 8MB + spikes 8MB and
the kernel is PE-bound, not DMA-bound.

GEMM: single-pass float32r. The PE rounds f32r inputs to ~fp22
(e10m11) with round-to-nearest; host pre-rounds to 11 mantissa bits
(measured bit-identical to the HW rounding - documents the precision
contract; rel err vs the fp32 reference is ~0.019 from threshold
flips). Measured f32r rates: 256-col moving streams at ~120ns/matmul
(the ~195ns 4-byte weight load mostly hides); 512-col moving is
slower per row - hence 256-col blocks. Per 8-timestep block, psum
holds [128, 8 chunks, 256 cols] = 4 banks, double-buffered across
blocks; chunks pack 2 per bank, so only the first chunk per bank may
issue start=True (the has_written clear is bank-wide).

Scan: z-space reformulation removes the per-step decay multiply. Host
prescales x columns of step tau (within a block) by s_tau = d^-(tau+1);
bias and the threshold shift enter through rank-1 bias-row matmuls
that OPEN each accumulation group (they only need two tiny tiles, so
block 0 computes while W streams). With the sign-flipped state
ybar = -d^-tau*(mem-1), one LIF step is 3 DVE ops:
    A: ybar -= G                     (psum read)
    B: spk_s = (ybar < 0) * s_tau    (dual-op tensor_scalar, fp32)
    C: ybar += spk_s
and ybar *= d^8 once per block. Spike pairs (two timesteps) DMA out on
the HW sync/scalar rings (gpsimd's software ring drains ~6us); host
binarizes the {0, s_tau} values exactly via != 0.
"""

import math

import numpy as np

import concourse.bacc as bacc
import concourse.mybir as mybir
import concourse.tile as tile
from concourse import bass_utils

# Problem constants (hardcoded per contract)
T, B, I, O = 64, 128, 2048, 2048
N_CORES = 8
OC_SHARD, BC_SHARD = 2, 4          # out_dim x batch sharding grid
O_LOC = O // OC_SHARD              # 1024 channels per core
B_LOC = B // BC_SHARD              # 32 batch rows per core
N_CHUNK = O_LOC // 128             # 8 stationary chunks
KT = I // 128                      # 16 k-tiles
STEPS_PER_BLK = 8                  # timesteps per psum block
N_BLK = T // STEPS_PER_BLK         # 8 blocks
COLS = STEPS_PER_BLK * B_LOC       # 256 moving columns per block
CHUNKS_PER_BANK = max(1, 512 // COLS)
TAU_C, THR = 2.0, 1.0
DECAY = math.exp(-1.0 / TAU_C)
SCALES = [DECAY ** -(t + 1) for t in range(STEPS_PER_BLK)]

F32 = mybir.dt.float32
F32R = mybir.dt.float32r
BF16 = mybir.dt.bfloat16
ALU = mybir.AluOpType

MODE = "f32r_o2b4v14"

_cache = {}


def _rne(a: np.ndarray, mant_bits: int = 11) -> np.ndarray:
    """Round fp32 array to mant_bits mantissa bits, round-to-nearest-even."""
    drop = 23 - mant_bits
    u = np.ascontiguousarray(a, dtype=np.float32).view(np.uint32)
    lsb = (u >> drop) & 1
    u = u + ((1 << (drop - 1)) - 1) + lsb
    u &= np.uint32(0xFFFFFFFF) ^ np.uint32((1 << drop) - 1)
    return u.view(np.float32)


def _build_nc():
    nc = bacc.Bacc(trn_type="TRN2", target_bir_lowering=False)

    # DRAM I/O (per core). x_packed[k, blk] is a contiguous [128, COLS]
    # tile: host-transposed, column-prescaled by s_tau, RNE-rounded.
    x_d = nc.dram_tensor("x_packed", [KT, N_BLK, 128, COLS], F32R,
                         kind="ExternalInput")
    w_d = nc.dram_tensor("w_packed", [128, KT, N_CHUNK, 128], F32R,
                         kind="ExternalInput")
    wb_d = nc.dram_tensor("wb", [1, N_CHUNK, 128], F32R, kind="ExternalInput")
    xb_d = nc.dram_tensor("xb", [1, COLS], F32R, kind="ExternalInput")
    out_d = nc.dram_tensor("out", [128, T, N_CHUNK, B_LOC], F32,
                           kind="ExternalOutput")

    with tile.TileContext(nc) as tc:
        with (
            tc.tile_pool(name="wpool", bufs=1) as wpool,
            tc.tile_pool(name="xpool", bufs=10) as xpool,
            tc.tile_pool(name="state", bufs=1) as state,
            tc.tile_pool(name="spkpool", bufs=4) as spkpool,
            tc.tile_pool(name="psum", bufs=2, space="PSUM") as psum_pool,
        ):
            # All bulk DMAs ride the sync+scalar HWDGE rings; gpsimd's
            # ring is software-DGE (slow ~6us drain) and its first DMA is
            # delayed ~10us by the global dma_reset/sem_clear preamble, so
            # it only carries W[k=8..11] (needed late). W streams k-wise with
            # block-0 x tiles (issued in the bi==0 loop below) so both
            # stay just ahead of GEMM consumption.
            w_all = wpool.tile([128, KT, N_CHUNK, 128], F32R)
            wb_t = wpool.tile([1, N_CHUNK, 128], F32R)
            nc.sync.dma_start(wb_t[:], wb_d[:])
            xb_t = wpool.tile([1, COLS], F32R)
            nc.scalar.dma_start(xb_t[:], xb_d[:])

            # State: ybar = -d^-tau (mem - 1); mem_0 = 0 -> ybar = 1.
            ybar = state.tile([128, N_CHUNK, B_LOC], F32)
            nc.vector.memset(ybar[:], 1.0)

            # Small PE pre-warm sized to the wb/xb DMA wait: starts the
            # HAM busy window so the real stream begins warm. bf16 so each
            # source matmul is a single HW matmul (fp32 lowers to 2).
            # Output lands in the first psum buffer and is wiped by the
            # real start=True matmuls.
            wsrc = state.tile([1, 128], BF16)
            nc.vector.memset(wsrc[:], 0.0)
            xsrc = state.tile([1, COLS], BF16)
            nc.vector.memset(xsrc[:], 0.0)
            warm = psum_pool.tile([128, N_CHUNK, COLS], F32, tag="ps",
                                  name="ps_warm")
            for i in range(22):
                nc.tensor.matmul(warm[:, 0, :], wsrc[:], xsrc[:],
                                 start=True, stop=(i == 21))

            for bi in range(N_BLK):
                ps = psum_pool.tile([128, N_CHUNK, COLS], F32, tag="ps",
                                    name=f"ps_{bi}")
                # Rank-1 bias row OPENS each chunk's accumulation group
                # (adds s_tau * (b_o + d - 1) to every column): it only
                # needs the tiny wb/xb tiles, so block 0 can start matmuls
                # ~5us before the first W slice lands. start=True clears
                # has_written for the WHOLE bank, so when chunks share a
                # bank only the first chunk in the bank may clear; the
                # others overwrite onto cleared bits.
                for c in range(N_CHUNK):
                    nc.tensor.matmul(
                        ps[:, c, :], wb_t[:, c, :], xb_t[:],
                        start=(c % CHUNKS_PER_BANK == 0), stop=False,
                    )
                for k in range(KT):
                    xt = xpool.tile([128, COLS], F32R, tag="xt",
                                    name=f"xt_{bi}_{k}")
                    eng = nc.sync if k % 2 == 0 else nc.scalar
                    if bi == 0:
                        # Block-0 feed is ~10MB against a ~17us GEMM: ride
                        # all three rings, items ordered by consumption
                        # deadline. HW rings (sync/scalar) carry W k<8 and
                        # k>=11 striped in chunk-halves plus the early x
                        # tiles; gpsimd (ring starts ~10.5us after its
                        # preamble) carries W[8..10] and the late x tiles
                        # k>=6, whose deadlines it comfortably makes.
                        if k in (8, 9, 10):
                            nc.gpsimd.dma_start(w_all[:, k], w_d[:, k])
                        else:
                            half = N_CHUNK // 2
                            nc.sync.dma_start(w_all[:, k, :half], w_d[:, k, :half])
                            nc.scalar.dma_start(w_all[:, k, half:], w_d[:, k, half:])
                        if k >= 6:
                            nc.gpsimd.dma_start(xt[:], x_d[k, bi])
                        else:
                            eng.dma_start(xt[:], x_d[k, bi])
                    else:
                        eng.dma_start(xt[:], x_d[k, bi])
                    for c in range(N_CHUNK):
                        nc.tensor.matmul(
                            ps[:, c, :], w_all[:, k, c, :], xt[:],
                            start=False, stop=(k == KT - 1),
                        )

                # LIF scan consuming this block's psum. Spikes for two
                # consecutive steps share one tile and go out in a single
                # DMA (out[:, t-1:t+1] is contiguous), halving descriptors.
                for tau in range(STEPS_PER_BLK):
                    t = bi * STEPS_PER_BLK + tau
                    g = ps[:, :, tau * B_LOC:(tau + 1) * B_LOC]
                    nc.vector.tensor_tensor(ybar[:], ybar[:], g,
                                            op=ALU.subtract)
                    if tau % 2 == 0:
                        # fp32: the {0, s_tau} reset values must carry the
                        # exact fp32 s_tau (bf16 rounding of s_tau injects
                        # ~2^-9 u-space error per spike -> flip blowup)
                        spk2 = spkpool.tile([128, 2, N_CHUNK, B_LOC], F32,
                                            tag="spk")
                    spk = spk2[:, tau % 2]
                    nc.vector.tensor_scalar(spk, ybar[:], 0.0, SCALES[tau],
                                            op0=ALU.is_lt, op1=ALU.mult)
                    if t + 1 < T:
                        nc.vector.tensor_tensor(ybar[:], ybar[:], spk,
                                                op=ALU.add)
                    if tau % 2 == 1:
                        eng = nc.sync if (t // 2) % 2 == 0 else nc.scalar
                        eng.dma_start(out_d[:, t - 1:t + 1], spk2[:])
                if bi + 1 < N_BLK:
                    nc.vector.tensor_scalar_mul(ybar[:], ybar[:],
                                                DECAY ** STEPS_PER_BLK)

    nc.compile()
    return nc


def _get_nc():
    if "nc" not in _cache:
        _cache["nc"] = _build_nc()
    return _cache["nc"]


def kernel(x_seq: np.ndarray, W: np.ndarray, b: np.ndarray) -> np.ndarray:
    nc = _get_nc()

    x_seq = np.ascontiguousarray(x_seq, dtype=np.float32)
    col_scale = np.array([SCALES[t % STEPS_PER_BLK] for t in range(T)],
                         dtype=np.float32)

    # Per-batch-shard x: [KT, N_BLK, 128, COLS], prescaled + RNE'd.
    x_parts = []
    for bc in range(BC_SHARD):
        xs = x_seq[:, bc * B_LOC:(bc + 1) * B_LOC, :]      # [T, B_LOC, I]
        xs = xs * col_scale[:, None, None]
        xp = xs.transpose(2, 0, 1)                         # [I, T, B_LOC]
        xp = xp.reshape(KT, 128, N_BLK, STEPS_PER_BLK * B_LOC)
        xp = np.ascontiguousarray(xp.transpose(0, 2, 1, 3))
        x_parts.append(_rne(xp))

    # Per-out-shard weights: [128(ip), KT, N_CHUNK, 128(of)], RNE'd.
    w_parts, wb_parts = [], []
    for oc in range(OC_SHARD):
        w_oc = W[oc * O_LOC:(oc + 1) * O_LOC, :].astype(np.float32)
        wT = _rne(w_oc.T)                                  # [I, O_LOC]
        wp = wT.reshape(KT, 128, N_CHUNK, 128).transpose(1, 0, 2, 3)
        w_parts.append(np.ascontiguousarray(wp))
        wb = b[oc * O_LOC:(oc + 1) * O_LOC].astype(np.float32) + DECAY - 1.0
        wb_parts.append(_rne(wb.reshape(1, N_CHUNK, 128)))

    xb = np.repeat(np.array(SCALES, dtype=np.float32), B_LOC).reshape(1, COLS)
    xb = _rne(xb)

    in_maps = []
    for c in range(N_CORES):
        oc, bc = divmod(c, BC_SHARD)
        in_maps.append({
            "x_packed": x_parts[bc],
            "w_packed": w_parts[oc],
            "wb": wb_parts[oc],
            "xb": xb,
        })

    res = bass_utils.run_bass_kernel_spmd(nc, in_maps, core_ids=list(range(N_CORES)))
    global LAST_RESULT
    LAST_RESULT = res

    # Assemble: out_c[op, t, chunk, beta] ({0, s_tau} fp32) -> {0,1} fp32
    result = np.empty((T, B, O), dtype=np.float32)
    for c in range(N_CORES):
        oc, bc = divmod(c, BC_SHARD)
        o_part = (res.results[c]["out"] != 0).astype(np.float32)
        part = o_part.transpose(1, 3, 2, 0).reshape(T, B_LOC, O_LOC)
        result[:, bc * B_LOC:(bc + 1) * B_LOC,
               oc * O_LOC:(oc + 1) * O_LOC] = part
    return result


LAST_RESULT = None
